# revision 2
# baseline (speedup 1.0000x reference)
"""MoE layer (8 experts, top-2, SwiGLU) on 8 Trainium2 NeuronCores.

Strategy (expert-parallel, per the sharding hint):
  - Host computes the tiny gate (0.07% of FLOPs) in float64 — this is the
    routing/dispatch metadata plus the load-balance loss scalar.
  - Tokens are dispatched to their top-2 experts; expert e's token set is
    gathered, transposed to [DIM, C] and shipped to core e.
  - Each core runs the full SwiGLU FFN for its expert in float32r (full-rate
    fp32 on the PE array) over its gathered tokens:
      phase A: hT = w1^T xT ; gT = silu(w2^T xT) ; hgT = hT*gT -> DRAM stage
      phase B: yT = w3^T hgT, scaled per-token by the combine weight.
  - Host scatter-adds the two weighted expert contributions per token.

All activations stay transposed ([feature, token]) so every matmul uses the
weights in their natural layout and no on-device transposes are needed.
"""

import os
import sys

for _p in ("/opt/trn_rl_repo", "/root/.axon_site/_ro/trn_rl_repo"):
    if os.path.isdir(_p) and _p not in sys.path:
        sys.path.insert(0, _p)

import numpy as np

DIM = 1024
HID = 2048
E = 8
TOPK = 2
INV_SQRT2 = 1.0 / 1.41421356237
P = 128
KO_D = DIM // P  # 8 k-subtiles for DIM contraction
KO_H = HID // P  # 16 k-subtiles for HID contraction
JCH = 4  # hidden chunks in phase A (HID/JCH = 512 wide)
HC = HID // JCH  # 512
MI_N = HC // P  # 4 psum m-subtiles per hidden chunk

_cache = {}


def _build_nc(blocks, C, use_b12):
    import concourse.mybir as mybir
    import concourse.tile as tile
    from concourse import bacc

    R = mybir.dt.float32r
    F = mybir.dt.float32

    nc = bacc.Bacc("TRN2", target_bir_lowering=False, debug=False, num_devices=E)

    xT = nc.dram_tensor("xT", [DIM, C], R, kind="ExternalInput")
    w1 = nc.dram_tensor("w1", [DIM, HID], R, kind="ExternalInput")
    w2 = nc.dram_tensor("w2", [DIM, HID], R, kind="ExternalInput")
    w3 = nc.dram_tensor("w3", [HID, DIM], R, kind="ExternalInput")
    wtok = nc.dram_tensor("wtok", [P, C], F, kind="ExternalInput")
    if use_b12:
        b1d = nc.dram_tensor("b1", [HID], F, kind="ExternalInput")
        b2d = nc.dram_tensor("b2", [HID], F, kind="ExternalInput")
    y = nc.dram_tensor("y", [DIM, C], F, kind="ExternalOutput")

    xT_t = xT.ap().rearrange("(ko p) c -> p ko c", p=P)
    w1_t = w1.ap().rearrange("(ko p) h -> p ko h", p=P)
    w2_t = w2.ap().rearrange("(ko p) h -> p ko h", p=P)
    w3_t = w3.ap().rearrange("(ko p) d -> p ko d", p=P)
    y_t = y.ap().rearrange("(mo p) c -> p mo c", p=P)

    with tile.TileContext(nc) as tc:
        with tc.tile_pool(name="dram", bufs=1, space="DRAM") as drampool:
            hg_stage = drampool.tile([P, KO_H, C], R)

            # ---------------- Phase A: hgT = (w1^T x^T) * silu(w2^T x^T)
            with (
                tc.tile_pool(name="wA", bufs=1) as wA,
                tc.tile_pool(name="xA", bufs=2) as xA,
                tc.tile_pool(name="workA", bufs=3) as workA,
                tc.tile_pool(name="psA", bufs=1, space="PSUM") as psA,
            ):
                w1_sb = []
                w2_sb = []
                for j in range(JCH):
                    t1 = wA.tile([P, KO_D, HC], R, tag=f"w1j{j}")
                    t2 = wA.tile([P, KO_D, HC], R, tag=f"w2j{j}")
                    for ko in range(KO_D):
                        nc.sync.dma_start(
                            t1[:, ko, :], w1_t[:, ko, j * HC : (j + 1) * HC]
                        )
                        nc.sync.dma_start(
                            t2[:, ko, :], w2_t[:, ko, j * HC : (j + 1) * HC]
                        )
                    w1_sb.append(t1)
                    w2_sb.append(t2)

                if use_b12:
                    b1_sb = wA.tile([P, KO_H], F, tag="b1")
                    b2_sb = wA.tile([P, KO_H], F, tag="b2")
                    nc.sync.dma_start(
                        b1_sb[:], b1d.ap().rearrange("(ko p) -> p ko", p=P)
                    )
                    nc.sync.dma_start(
                        b2_sb[:], b2d.ap().rearrange("(ko p) -> p ko", p=P)
                    )

                for c0, bs in blocks:
                    xb = xA.tile([P, KO_D, 512], R, tag="xb")
                    for ko in range(KO_D):
                        nc.sync.dma_start(
                            xb[:, ko, :bs], xT_t[:, ko, c0 : c0 + bs]
                        )
                    for j in range(JCH):
                        for mi in range(MI_N):
                            hs = slice(mi * P, (mi + 1) * P)
                            h_ps = psA.tile([P, 512], F, tag=f"h{mi}")
                            g_ps = psA.tile([P, 512], F, tag=f"g{mi}")
                            for ki in range(KO_D):
                                nc.tensor.matmul(
                                    h_ps[:, :bs],
                                    w1_sb[j][:, ki, hs],
                                    xb[:, ki, :bs],
                                    start=(ki == 0),
                                    stop=(ki == KO_D - 1),
                                )
                            for ki in range(KO_D):
                                nc.tensor.matmul(
                                    g_ps[:, :bs],
                                    w2_sb[j][:, ki, hs],
                                    xb[:, ki, :bs],
                                    start=(ki == 0),
                                    stop=(ki == KO_D - 1),
                                )
                            hidx = j * MI_N + mi
                            gact = workA.tile([P, 512], F, tag="gact")
                            if use_b12:
                                nc.scalar.activation(
                                    gact[:, :bs],
                                    g_ps[:, :bs],
                                    mybir.ActivationFunctionType.Silu,
                                    bias=b2_sb[:, hidx : hidx + 1],
                                )
                                nc.vector.tensor_scalar_add(
                                    h_ps[:, :bs],
                                    h_ps[:, :bs],
                                    b1_sb[:, hidx : hidx + 1],
                                )
                            else:
                                nc.scalar.activation(
                                    gact[:, :bs],
                                    g_ps[:, :bs],
                                    mybir.ActivationFunctionType.Silu,
                                )
                            hg = workA.tile([P, 512], R, tag="hg")
                            nc.vector.tensor_mul(
                                hg[:, :bs], h_ps[:, :bs], gact[:, :bs]
                            )
                            nc.sync.dma_start(
                                hg_stage[:, hidx, c0 : c0 + bs], hg[:, :bs]
                            )

            # ---------------- Phase B: yT = w3^T hgT, scaled by wtok
            with (
                tc.tile_pool(name="wB", bufs=1) as wB,
                tc.tile_pool(name="hgB", bufs=2) as hgB,
                tc.tile_pool(name="outB", bufs=4) as outB,
                tc.tile_pool(name="psB", bufs=1, space="PSUM") as psB,
            ):
                w3_sb = wB.tile([P, KO_H, DIM], R, tag="w3")
                for ko in range(KO_H):
                    nc.sync.dma_start(w3_sb[:, ko, :], w3_t[:, ko, :])
                wtok_sb = wB.tile([P, C], F, tag="wtok")
                nc.sync.dma_start(wtok_sb[:], wtok.ap())

                for c0, bs in blocks:
                    hgb = hgB.tile([P, KO_H, 512], R, tag="hgb")
                    for ko in range(KO_H):
                        nc.sync.dma_start(
                            hgb[:, ko, :bs], hg_stage[:, ko, c0 : c0 + bs]
                        )
                    for mo in range(DIM // P):
                        y_ps = psB.tile([P, 512], F, tag=f"y{mo}")
                        for ki in range(KO_H):
                            nc.tensor.matmul(
                                y_ps[:, :bs],
                                w3_sb[:, ki, mo * P : (mo + 1) * P],
                                hgb[:, ki, :bs],
                                start=(ki == 0),
                                stop=(ki == KO_H - 1),
                            )
                        y_sb = outB.tile([P, 512], F, tag="ysb")
                        nc.vector.tensor_mul(
                            y_sb[:, :bs], y_ps[:, :bs], wtok_sb[:, c0 : c0 + bs]
                        )
                        nc.sync.dma_start(y_t[:, mo, c0 : c0 + bs], y_sb[:, :bs])

    nc.compile()
    return nc


def _get_nc(blocks, C, use_b12):
    key = (tuple(blocks), C, use_b12)
    if key not in _cache:
        _cache[key] = _build_nc(blocks, C, use_b12)
    return _cache[key]


def kernel(x, gate_w, gate_b, w1, b1, w2, b2, w3, b3):
    from concourse.bass_utils import run_bass_kernel_spmd

    x = np.asarray(x, dtype=np.float32)
    gate_w = np.asarray(gate_w, dtype=np.float32)
    gate_b = np.asarray(gate_b, dtype=np.float32)
    w1 = np.asarray(w1, dtype=np.float32)
    b1 = np.asarray(b1, dtype=np.float32)
    w2 = np.asarray(w2, dtype=np.float32)
    b2 = np.asarray(b2, dtype=np.float32)
    w3 = np.asarray(w3, dtype=np.float32)
    b3 = np.asarray(b3, dtype=np.float32)

    B, S, D = x.shape
    N = B * S
    xf = x.reshape(N, D)

    # ---- Gating / routing metadata (float64 so top-k decisions are exact)
    logits = xf.astype(np.float64) @ gate_w.astype(np.float64) + gate_b
    logits -= logits.max(axis=-1, keepdims=True)
    sc = np.exp(logits)
    sc /= sc.sum(axis=-1, keepdims=True)
    order = np.argsort(-sc, axis=-1, kind="stable")
    idx = order[:, :TOPK]  # [N, 2]
    s = np.take_along_axis(sc, idx, axis=-1)  # [N, 2]

    usage = sc.mean(axis=0)
    lb_loss = np.float32(-(usage * np.log(usage + 1e-9)).sum())

    # ---- Dispatch: gather each expert's tokens
    tok_lists = []
    wt_lists = []
    for e in range(E):
        toks = []
        wts = []
        for k in range(TOPK):
            m = idx[:, k] == e
            toks.append(np.nonzero(m)[0])
            wts.append(s[m, k])
        tok_lists.append(np.concatenate(toks))
        wt_lists.append(np.concatenate(wts).astype(np.float32))

    cmax = max(len(t) for t in tok_lists)
    C = max(256, -(-cmax // 256) * 256)
    blocks = []
    off = 0
    while off < C:
        bs = 512 if C - off >= 512 else 256
        blocks.append((off, bs))
        off += bs

    use_b12 = bool(np.any(b1) or np.any(b2))
    nc = _get_nc(blocks, C, use_b12)

    in_maps = []
    for e in range(E):
        toks = tok_lists[e]
        ce = len(toks)
        xTg = np.zeros((DIM, C), dtype=np.float32)
        xTg[:, :ce] = xf[toks].T
        wtok = np.zeros((C,), dtype=np.float32)
        wtok[:ce] = wt_lists[e] * np.float32(INV_SQRT2)
        m = {
            "xT": xTg,
            "w1": np.ascontiguousarray(w1[e]),
            "w2": np.ascontiguousarray(w2[e]),
            "w3": np.ascontiguousarray(w3[e]),
            "wtok": np.ascontiguousarray(np.broadcast_to(wtok, (P, C))),
        }
        if use_b12:
            m["b1"] = np.ascontiguousarray(b1[e])
            m["b2"] = np.ascontiguousarray(b2[e])
        in_maps.append(m)

    res = run_bass_kernel_spmd(nc, in_maps, list(range(E))).results

    # ---- Combine: out[t] = sum over t's two experts of weighted outputs
    outT = np.zeros((DIM, N), dtype=np.float32)
    for e in range(E):
        toks = tok_lists[e]
        outT[:, toks] += res[e]["y"][:, : len(toks)]

    if np.any(b3):
        bsel = (
            b3[idx[:, 0]] * (s[:, 0:1] * INV_SQRT2)
            + b3[idx[:, 1]] * (s[:, 1:2] * INV_SQRT2)
        ).astype(np.float32)  # [N, DIM]
        outT += bsel.T

    out = np.ascontiguousarray(outT.T).reshape(B, S, D)
    return out, lb_loss


# revision 6
# speedup vs baseline: 1.1130x; 1.1130x over previous
"""MoE layer (8 experts, top-2, SwiGLU) on 8 Trainium2 NeuronCores.

Strategy (expert-parallel, per the sharding hint):
  - Host computes the tiny gate (0.07% of FLOPs) in float64 — this is the
    routing/dispatch metadata plus the load-balance loss scalar.
  - Tokens are dispatched to their top-2 experts; expert e's token set is
    gathered, transposed to [DIM, C] and shipped to core e.
  - Each core runs the full SwiGLU FFN for its expert in float32r (full-rate
    fp32 on the PE array) over its gathered tokens:
      phase A: hT = w1^T xT ; gT = silu(w2^T xT) ; hgT = hT*gT -> DRAM stage
      phase B: yT = w3^T hgT, scaled per-token by the combine weight.
  - Host scatter-adds the two weighted expert contributions per token.

All activations stay transposed ([feature, token]) so every matmul uses the
weights in their natural layout and no on-device transposes are needed.
"""

import os
import sys

for _p in ("/opt/trn_rl_repo", "/root/.axon_site/_ro/trn_rl_repo"):
    if os.path.isdir(_p) and _p not in sys.path:
        sys.path.insert(0, _p)

import numpy as np

DIM = 1024
HID = 2048
E = 8
TOPK = 2
INV_SQRT2 = 1.0 / 1.41421356237
P = 128
KO_D = DIM // P  # 8 k-subtiles for DIM contraction
KO_H = HID // P  # 16 k-subtiles for HID contraction
JCH = 4  # hidden chunks in phase A (HID/JCH = 512 wide)
HC = HID // JCH  # 512
MI_N = HC // P  # 4 psum m-subtiles per hidden chunk

_cache = {}


def _build_nc(blocks, C, use_b12):
    import concourse.mybir as mybir
    import concourse.tile as tile
    from concourse import bacc

    R = mybir.dt.float32r
    F = mybir.dt.float32

    nc = bacc.Bacc("TRN2", target_bir_lowering=False, debug=False, num_devices=E)

    xT = nc.dram_tensor("xT", [DIM, C], R, kind="ExternalInput")
    w1 = nc.dram_tensor("w1", [DIM, HID], R, kind="ExternalInput")
    w2 = nc.dram_tensor("w2", [DIM, HID], R, kind="ExternalInput")
    w3 = nc.dram_tensor("w3", [HID, DIM], R, kind="ExternalInput")
    wtok = nc.dram_tensor("wtok", [P, C], F, kind="ExternalInput")
    if use_b12:
        b1d = nc.dram_tensor("b1", [HID], F, kind="ExternalInput")
        b2d = nc.dram_tensor("b2", [HID], F, kind="ExternalInput")
    y = nc.dram_tensor("y", [DIM, C], F, kind="ExternalOutput")

    xT_t = xT.ap().rearrange("(ko p) c -> p ko c", p=P)
    w1_t = w1.ap().rearrange("(ko p) h -> p ko h", p=P)
    w2_t = w2.ap().rearrange("(ko p) h -> p ko h", p=P)
    w3_t = w3.ap().rearrange("(ko p) d -> p ko d", p=P)
    y_t = y.ap().rearrange("(mo p) c -> p mo c", p=P)

    W3PRE = 6  # w3 k-chunks prefetched into spare SBUF during phase A

    with tile.TileContext(nc) as tc:
        with (
            tc.tile_pool(name="dram", bufs=1, space="DRAM") as drampool,
            tc.tile_pool(name="w3pre", bufs=1) as w3pre,
        ):
            hg_stage = drampool.tile([P, KO_H, C], R)

            w3_sb = [None] * KO_H
            wtok_sb = None

            # ---------------- Phase A: hgT = (w1^T x^T) * silu(w2^T x^T)
            with (
                tc.tile_pool(name="wA", bufs=1) as wA,
                tc.tile_pool(name="xA", bufs=2) as xA,
                tc.tile_pool(name="workA", bufs=3) as workA,
                tc.tile_pool(name="psA", bufs=1, space="PSUM") as psA,
            ):
                def load_xblock(c0, bs):
                    tiles = []
                    for ko in range(KO_D):
                        t = xA.tile([P, 512], R, tag=f"xb{ko}", name=f"xb{ko}")
                        nc.sync.dma_start(t[:, :bs], xT_t[:, ko, c0 : c0 + bs])
                        tiles.append(t)
                    return tiles

                # Block-0 activations first so the first matmuls start early.
                xb0 = load_xblock(*blocks[0])

                # Weights: j0 chunks first (first compute wave), rest behind.
                w1_sb = [[None] * KO_D for _ in range(JCH)]
                w2_sb = [[None] * KO_D for _ in range(JCH)]
                for j in range(JCH):
                    for ko in range(KO_D):
                        t1 = wA.tile([P, HC], R, tag=f"w1j{j}k{ko}", name=f"w1j{j}k{ko}")
                        nc.sync.dma_start(t1[:], w1_t[:, ko, j * HC : (j + 1) * HC])
                        t2 = wA.tile([P, HC], R, tag=f"w2j{j}k{ko}", name=f"w2j{j}k{ko}")
                        nc.sync.dma_start(t2[:], w2_t[:, ko, j * HC : (j + 1) * HC])
                        w1_sb[j][ko] = t1
                        w2_sb[j][ko] = t2

                if use_b12:
                    b1_sb = wA.tile([P, KO_H], F, tag="b1")
                    b2_sb = wA.tile([P, KO_H], F, tag="b2")
                    nc.sync.dma_start(
                        b1_sb[:], b1d.ap().rearrange("(ko p) -> p ko", p=P)
                    )
                    nc.sync.dma_start(
                        b2_sb[:], b2d.ap().rearrange("(ko p) -> p ko", p=P)
                    )

                for bi_, (c0, bs) in enumerate(blocks):
                    xb = xb0 if bi_ == 0 else load_xblock(c0, bs)
                    for j in range(JCH):
                        for mi in range(MI_N):
                            hs = slice(mi * P, (mi + 1) * P)
                            h_ps = psA.tile([P, 512], F, tag=f"h{mi}")
                            g_ps = psA.tile([P, 512], F, tag=f"g{mi}")
                            for ki in range(KO_D):
                                nc.tensor.matmul(
                                    h_ps[:, :bs],
                                    w1_sb[j][ki][:, hs],
                                    xb[ki][:, :bs],
                                    start=(ki == 0),
                                    stop=(ki == KO_D - 1),
                                )
                            for ki in range(KO_D):
                                nc.tensor.matmul(
                                    g_ps[:, :bs],
                                    w2_sb[j][ki][:, hs],
                                    xb[ki][:, :bs],
                                    start=(ki == 0),
                                    stop=(ki == KO_D - 1),
                                )
                            hidx = j * MI_N + mi
                            gact = workA.tile([P, 512], F, tag="gact")
                            if use_b12:
                                nc.scalar.activation(
                                    gact[:, :bs],
                                    g_ps[:, :bs],
                                    mybir.ActivationFunctionType.Silu,
                                    bias=b2_sb[:, hidx : hidx + 1],
                                )
                                nc.vector.tensor_scalar_add(
                                    h_ps[:, :bs],
                                    h_ps[:, :bs],
                                    b1_sb[:, hidx : hidx + 1],
                                )
                            else:
                                nc.scalar.activation(
                                    gact[:, :bs],
                                    g_ps[:, :bs],
                                    mybir.ActivationFunctionType.Silu,
                                )
                            hg = workA.tile([P, 512], R, tag="hg")
                            nc.vector.tensor_mul(
                                hg[:, :bs], h_ps[:, :bs], gact[:, :bs]
                            )
                            nc.sync.dma_start(
                                hg_stage[:, hidx, c0 : c0 + bs], hg[:, :bs]
                            )

                # Prefetch half of w3 + wtok into spare SBUF while phase A
                # drains (low priority: emitted last).
                for ko in range(W3PRE):
                    t = w3pre.tile([P, DIM], R, tag=f"w3k{ko}", name=f"w3k{ko}")
                    nc.sync.dma_start(t[:], w3_t[:, ko, :])
                    w3_sb[ko] = t

            # ---------------- Phase B: yT = w3^T hgT, scaled by wtok
            with (
                tc.tile_pool(name="wB", bufs=1) as wB,
                tc.tile_pool(name="hgB", bufs=2) as hgB,
                tc.tile_pool(name="outB", bufs=4) as outB,
                tc.tile_pool(name="psB", bufs=1, space="PSUM") as psB,
            ):
                wtok_sb = wB.tile([P, C], F, tag="wtok")
                nc.sync.dma_start(wtok_sb[:], wtok.ap())
                for ko in range(W3PRE, KO_H):
                    t = wB.tile([P, DIM], R, tag=f"w3k{ko}", name=f"w3k{ko}")
                    nc.sync.dma_start(t[:], w3_t[:, ko, :])
                    w3_sb[ko] = t

                for c0, bs in blocks:
                    hgb = []
                    for ko in range(KO_H):
                        t = hgB.tile([P, 512], R, tag=f"hgb{ko}", name=f"hgb{ko}")
                        nc.sync.dma_start(
                            t[:, :bs], hg_stage[:, ko, c0 : c0 + bs]
                        )
                        hgb.append(t)
                    for mo in range(DIM // P):
                        y_ps = psB.tile([P, 512], F, tag=f"y{mo}")
                        for ki in range(KO_H):
                            nc.tensor.matmul(
                                y_ps[:, :bs],
                                w3_sb[ki][:, mo * P : (mo + 1) * P],
                                hgb[ki][:, :bs],
                                start=(ki == 0),
                                stop=(ki == KO_H - 1),
                            )
                        y_sb = outB.tile([P, 512], F, tag="ysb")
                        nc.vector.tensor_mul(
                            y_sb[:, :bs], y_ps[:, :bs], wtok_sb[:, c0 : c0 + bs]
                        )
                        nc.sync.dma_start(y_t[:, mo, c0 : c0 + bs], y_sb[:, :bs])

    nc.compile()
    return nc


def _get_nc(blocks, C, use_b12):
    key = (tuple(blocks), C, use_b12)
    if key not in _cache:
        _cache[key] = _build_nc(blocks, C, use_b12)
    return _cache[key]


def kernel(x, gate_w, gate_b, w1, b1, w2, b2, w3, b3):
    from concourse.bass_utils import run_bass_kernel_spmd

    x = np.asarray(x, dtype=np.float32)
    gate_w = np.asarray(gate_w, dtype=np.float32)
    gate_b = np.asarray(gate_b, dtype=np.float32)
    w1 = np.asarray(w1, dtype=np.float32)
    b1 = np.asarray(b1, dtype=np.float32)
    w2 = np.asarray(w2, dtype=np.float32)
    b2 = np.asarray(b2, dtype=np.float32)
    w3 = np.asarray(w3, dtype=np.float32)
    b3 = np.asarray(b3, dtype=np.float32)

    B, S, D = x.shape
    N = B * S
    xf = x.reshape(N, D)

    # ---- Gating / routing metadata (float64 so top-k decisions are exact)
    logits = xf.astype(np.float64) @ gate_w.astype(np.float64) + gate_b
    logits -= logits.max(axis=-1, keepdims=True)
    sc = np.exp(logits)
    sc /= sc.sum(axis=-1, keepdims=True)
    order = np.argsort(-sc, axis=-1, kind="stable")
    idx = order[:, :TOPK]  # [N, 2]
    s = np.take_along_axis(sc, idx, axis=-1)  # [N, 2]

    usage = sc.mean(axis=0)
    lb_loss = np.float32(-(usage * np.log(usage + 1e-9)).sum())

    # ---- Dispatch: gather each expert's tokens
    tok_lists = []
    wt_lists = []
    for e in range(E):
        toks = []
        wts = []
        for k in range(TOPK):
            m = idx[:, k] == e
            toks.append(np.nonzero(m)[0])
            wts.append(s[m, k])
        tok_lists.append(np.concatenate(toks))
        wt_lists.append(np.concatenate(wts).astype(np.float32))

    cmax = max(len(t) for t in tok_lists)
    C = max(256, -(-cmax // 256) * 256)
    blocks = []
    off = 0
    while off < C:
        bs = 512 if C - off >= 512 else 256
        blocks.append((off, bs))
        off += bs

    use_b12 = bool(np.any(b1) or np.any(b2))
    nc = _get_nc(blocks, C, use_b12)

    in_maps = []
    for e in range(E):
        toks = tok_lists[e]
        ce = len(toks)
        xTg = np.zeros((DIM, C), dtype=np.float32)
        xTg[:, :ce] = xf[toks].T
        wtok = np.zeros((C,), dtype=np.float32)
        wtok[:ce] = wt_lists[e] * np.float32(INV_SQRT2)
        m = {
            "xT": xTg,
            "w1": np.ascontiguousarray(w1[e]),
            "w2": np.ascontiguousarray(w2[e]),
            "w3": np.ascontiguousarray(w3[e]),
            "wtok": np.ascontiguousarray(np.broadcast_to(wtok, (P, C))),
        }
        if use_b12:
            m["b1"] = np.ascontiguousarray(b1[e])
            m["b2"] = np.ascontiguousarray(b2[e])
        in_maps.append(m)

    res = run_bass_kernel_spmd(nc, in_maps, list(range(E))).results

    # ---- Combine: out[t] = sum over t's two experts of weighted outputs
    outT = np.zeros((DIM, N), dtype=np.float32)
    for e in range(E):
        toks = tok_lists[e]
        outT[:, toks] += res[e]["y"][:, : len(toks)]

    if np.any(b3):
        bsel = (
            b3[idx[:, 0]] * (s[:, 0:1] * INV_SQRT2)
            + b3[idx[:, 1]] * (s[:, 1:2] * INV_SQRT2)
        ).astype(np.float32)  # [N, DIM]
        outT += bsel.T

    out = np.ascontiguousarray(outT.T).reshape(B, S, D)
    return out, lb_loss
